# revision 7
# baseline (speedup 1.0000x reference)
"""BinaryLinear Trainium2 kernel: Y = X @ binarize(W).T + bias.

Shapes (hardcoded per the problem spec):
  X: [8192, 4096] f32, W: [4096, 4096] f32, bias: [4096] f32 -> Y: [8192, 4096] f32

Strategy: data-parallel over tokens across 8 NeuronCores (1024 tokens/core),
weight replicated. Host prepares transposed layouts (X.T shard and W.T) so the
contraction dim lands on SBUF partitions; all math (binarize + matmul + bias)
runs on-device.

Per core: X^T shard is made resident in SBUF (rounded to the compute dtype),
W^T streams through once; 2048 matmuls of [128x128]@[128x512] accumulate over
K=4096 into 8 PSUM banks per 512-wide out-block. X staging is interleaved into
the first out-block's k-loop so the PE starts ~10us in. Casts/drains run on the
otherwise-idle ACT engine; binarize + bias-add on DVE.

Compute modes (env TRNKERNEL_MODE):
  f32r   (default): fp32r matmuls — full-rate reduced-precision fp32
  bf16   : single-pass bf16 (X rounded to bf16)
  bf16x2 : X split into hi+lo bf16, two accumulating passes (near-fp32 exact)
"""
import os
import sys

import numpy as np

sys.path.insert(0, "/opt/trn_rl_repo")

import concourse.bacc as bacc
import concourse.mybir as mybir
import concourse.tile as tile
from concourse.bass_utils import run_bass_kernel_spmd

N_TOKENS = 8192
IN_F = 4096
OUT_F = 4096
N_CORES = 8
TOK_C = N_TOKENS // N_CORES  # 1024 tokens per core

P = 128
K_TILES = IN_F // P          # 32
KG = 4                       # k-tiles per W DMA/binarize group
K_GROUPS = K_TILES // KG     # 8
M_TILES = TOK_C // P         # 8
OB = 512                     # out-features per block (one PSUM bank)
O_BLOCKS = OUT_F // OB       # 8
XKG = 2                      # k-tiles per X-load DMA (1 MiB)

_MODE = os.environ.get("TRNKERNEL_MODE", "f32r")
_TRACE = os.environ.get("TRNKERNEL_TRACE", "0") == "1"

_CACHED = {}


def _install_ntff_shim():
    """Register the NTFF profile hook so trace=True yields exec_time_ns."""
    import types

    try:
        import antenv  # noqa: F401
        from trn_agent_boot.trn_boot import _ntff_profile_via_ctypes
        import concourse.bass_utils as bu

        hook = _ntff_profile_via_ctypes("/opt/axon/libaxon_pjrt.so")
        mod = types.ModuleType("antenv.axon_hooks")
        mod.get_axon_ntff_profile_hook = lambda: hook
        mod.set_axon_ntff_profile_hook = lambda h: None
        sys.modules["antenv.axon_hooks"] = mod
        bu.upload_artifacts = lambda tmpdir: tmpdir  # no artifact store here
    except Exception:
        pass


def build(mode: str):
    assert mode in ("f32r", "bf16", "bf16x2")
    mm_dt = mybir.dt.float32r if mode == "f32r" else mybir.dt.bfloat16

    nc = bacc.Bacc(None)
    xt = nc.declare_dram_parameter("xt", [IN_F, TOK_C], mybir.dt.float32, isOutput=False)
    wt = nc.declare_dram_parameter("wt", [IN_F, OUT_F], mybir.dt.float32, isOutput=False)
    bias = nc.declare_dram_parameter("bias", [OUT_F], mybir.dt.float32, isOutput=False)
    y = nc.declare_dram_parameter("y", [TOK_C, OUT_F], mybir.dt.float32, isOutput=True)

    # DRAM-side tiled views: partition dim = contraction (in-features)
    xt_v = xt.rearrange("(kt p) t -> p kt t", p=P)      # [128, 32, 1024]
    wt_v = wt.rearrange("(kt p) o -> p kt o", p=P)      # [128, 32, 4096]
    y_v = y.rearrange("(mt p) o -> p mt o", p=P)        # [128, 8, 4096]

    n_x = 2 if mode == "bf16x2" else 1

    with tile.TileContext(nc) as tc:
        with (
            tc.tile_pool(name="xres", bufs=1) as xres_pool,
            tc.tile_pool(name="xstage", bufs=2) as xstage_pool,
            tc.tile_pool(name="wstage", bufs=3) as wstage_pool,
            tc.tile_pool(name="wb", bufs=3) as wb_pool,
            tc.tile_pool(name="biasp", bufs=1) as bias_pool,
            tc.tile_pool(name="osb", bufs=4) as osb_pool,
            tc.tile_pool(name="psum", bufs=1, space="PSUM") as psum_pool,
        ):
            xr = [
                xres_pool.tile([P, K_TILES, TOK_C], mm_dt, tag=f"xr{i}", name=f"xr{i}")
                for i in range(n_x)
            ]

            def load_x_chunk(kk):
                """DMA one [128, XKG, 1024] X^T chunk and round into xr (ACT)."""
                xs = xstage_pool.tile([P, XKG, TOK_C], mybir.dt.float32, name="xs")
                nc.sync.dma_start(out=xs[:], in_=xt_v[:, kk * XKG:(kk + 1) * XKG, :])
                sl = slice(kk * XKG, (kk + 1) * XKG)
                nc.scalar.copy(out=xr[0][:, sl, :], in_=xs[:])
                if mode == "bf16x2":
                    nc.vector.tensor_sub(out=xr[1][:, sl, :], in0=xs[:], in1=xr[0][:, sl, :])

            for ob in range(O_BLOCKS):
                osl = slice(ob * OB, (ob + 1) * OB)

                # bias for this out-block, broadcast across partitions (ACT copy
                # so the DVE bias-add waits on a single semaphore)
                bstage = bias_pool.tile([P, OB], mybir.dt.float32, tag="bstage", name="bstage")
                nc.scalar.dma_start(out=bstage[:], in_=bias[None, osl].to_broadcast([P, OB]))
                bias_bc = bias_pool.tile([P, OB], mybir.dt.float32, tag="bbc", name="bias_bc")
                nc.scalar.copy(out=bias_bc[:], in_=bstage[:])

                psums = [psum_pool.tile([P, OB], mybir.dt.float32, name=f"ps{_m}") for _m in range(M_TILES)]

                for kg in range(K_GROUPS):
                    if ob == 0:
                        # interleave X residency build into the first out-block
                        for kk in range(kg * KG // XKG, (kg + 1) * KG // XKG):
                            load_x_chunk(kk)
                    ws = wstage_pool.tile([P, KG, OB], mybir.dt.float32, name="ws")
                    nc.sync.dma_start(out=ws[:], in_=wt_v[:, kg * KG:(kg + 1) * KG, osl])
                    wb = wb_pool.tile([P, KG, OB], mm_dt, name="wb")
                    nc.vector.tensor_scalar(
                        out=wb[:], in0=ws[:], scalar1=0.0, scalar2=None,
                        op0=mybir.AluOpType.is_gt,
                    )
                    for ks in range(KG):
                        k = kg * KG + ks
                        for m in range(M_TILES):
                            nc.tensor.matmul(
                                out=psums[m][:],
                                lhsT=xr[0][:, k, m * P:(m + 1) * P],
                                rhs=wb[:, ks, :],
                                start=(k == 0),
                                stop=(k == K_TILES - 1) if mode != "bf16x2" else False,
                            )
                            if mode == "bf16x2":
                                nc.tensor.matmul(
                                    out=psums[m][:],
                                    lhsT=xr[1][:, k, m * P:(m + 1) * P],
                                    rhs=wb[:, ks, :],
                                    start=False,
                                    stop=(k == K_TILES - 1),
                                )

                # drain: psum -> sbuf (ACT), + bias (DVE), -> DRAM
                for m in range(M_TILES):
                    o_sb = osb_pool.tile([P, OB], mybir.dt.float32, name="o_sb")
                    nc.scalar.copy(out=o_sb[:], in_=psums[m][:])
                    nc.vector.tensor_add(out=o_sb[:], in0=o_sb[:], in1=bias_bc[:])
                    nc.scalar.dma_start(out=y_v[:, m, osl], in_=o_sb[:])

    nc.compile()
    return nc


def kernel(X: np.ndarray, weight: np.ndarray, bias: np.ndarray) -> np.ndarray:
    assert X.shape == (N_TOKENS, IN_F) and weight.shape == (OUT_F, IN_F)
    mode = _MODE

    if mode not in _CACHED:
        _CACHED[mode] = build(mode)
    nc = _CACHED[mode]

    if _TRACE:
        _install_ntff_shim()

    # Host-side layout prep (sharding + transposes only; math is on-device)
    wt_np = np.ascontiguousarray(weight.T.astype(np.float32, copy=False))
    bias_np = np.ascontiguousarray(bias.astype(np.float32, copy=False))
    in_maps = []
    for c in range(N_CORES):
        xs = X[c * TOK_C:(c + 1) * TOK_C, :]
        xt_np = np.ascontiguousarray(xs.T.astype(np.float32, copy=False))
        in_maps.append({"xt": xt_np, "wt": wt_np, "bias": bias_np})

    res = run_bass_kernel_spmd(
        nc, in_maps, core_ids=list(range(N_CORES)), trace=_TRACE,
    )
    out = np.concatenate([res.results[c]["y"] for c in range(N_CORES)], axis=0)
    if _TRACE:
        kernel.last_exec_time_ns = res.exec_time_ns
        kernel.last_trace = res.instructions_and_trace
    return out.astype(np.float32, copy=False)


# revision 8
# speedup vs baseline: 1.0470x; 1.0470x over previous
"""BinaryLinear Trainium2 kernel: Y = X @ binarize(W).T + bias.

Shapes (hardcoded per the problem spec):
  X: [8192, 4096] f32, W: [4096, 4096] f32, bias: [4096] f32 -> Y: [8192, 4096] f32

Strategy: data-parallel over tokens across 8 NeuronCores (1024 tokens/core),
weight replicated. Host prepares transposed layouts (X.T shard and W.T) so the
contraction dim lands on SBUF partitions; all math (binarize + matmul + bias)
runs on-device.

Per core: X^T shard is made resident in SBUF (rounded to the compute dtype),
W^T streams through once; 2048 matmuls of [128x128]@[128x512] accumulate over
K=4096 into 8 PSUM banks per 512-wide out-block. X staging is interleaved into
the first out-block's k-loop so the PE starts ~10us in. Casts/drains run on the
otherwise-idle ACT engine; binarize + bias-add on DVE.

Compute modes (env TRNKERNEL_MODE):
  f32r   (default): fp32r matmuls — full-rate reduced-precision fp32
  bf16   : single-pass bf16 (X rounded to bf16)
  bf16x2 : X split into hi+lo bf16, two accumulating passes (near-fp32 exact)
"""
import os
import sys

import numpy as np

sys.path.insert(0, "/opt/trn_rl_repo")

import concourse.bacc as bacc
import concourse.mybir as mybir
import concourse.tile as tile
from concourse.bass_utils import run_bass_kernel_spmd

N_TOKENS = 8192
IN_F = 4096
OUT_F = 4096
N_CORES = 8
TOK_C = N_TOKENS // N_CORES  # 1024 tokens per core

P = 128
K_TILES = IN_F // P          # 32
KG = 4                       # k-tiles per W DMA/binarize group
K_GROUPS = K_TILES // KG     # 8
M_TILES = TOK_C // P         # 8
OB = 512                     # out-features per block (one PSUM bank)
O_BLOCKS = OUT_F // OB       # 8
XKG = 2                      # k-tiles per X-load DMA (1 MiB)

_MODE = os.environ.get("TRNKERNEL_MODE", "f32r")
_TRACE = os.environ.get("TRNKERNEL_TRACE", "0") == "1"

_CACHED = {}


def _install_ntff_shim():
    """Register the NTFF profile hook so trace=True yields exec_time_ns."""
    import types

    try:
        import antenv  # noqa: F401
        from trn_agent_boot.trn_boot import _ntff_profile_via_ctypes
        import concourse.bass_utils as bu

        hook = _ntff_profile_via_ctypes("/opt/axon/libaxon_pjrt.so")
        mod = types.ModuleType("antenv.axon_hooks")
        mod.get_axon_ntff_profile_hook = lambda: hook
        mod.set_axon_ntff_profile_hook = lambda h: None
        sys.modules["antenv.axon_hooks"] = mod
        bu.upload_artifacts = lambda tmpdir: tmpdir  # no artifact store here
    except Exception:
        pass


def build(mode: str):
    assert mode in ("f32r", "bf16", "bf16x2")
    mm_dt = mybir.dt.float32r if mode == "f32r" else mybir.dt.bfloat16

    nc = bacc.Bacc(None)
    xt = nc.declare_dram_parameter("xt", [IN_F, TOK_C], mybir.dt.float32, isOutput=False)
    wt = nc.declare_dram_parameter("wt", [IN_F, OUT_F], mybir.dt.float32, isOutput=False)
    bias = nc.declare_dram_parameter("bias", [OUT_F], mybir.dt.float32, isOutput=False)
    y = nc.declare_dram_parameter("y", [TOK_C, OUT_F], mybir.dt.float32, isOutput=True)

    # DRAM-side tiled views: partition dim = contraction (in-features)
    xt_v = xt.rearrange("(kt p) t -> p kt t", p=P)      # [128, 32, 1024]
    wt_v = wt.rearrange("(kt p) o -> p kt o", p=P)      # [128, 32, 4096]
    y_v = y.rearrange("(mt p) o -> p mt o", p=P)        # [128, 8, 4096]

    n_x = 2 if mode == "bf16x2" else 1

    with tile.TileContext(nc) as tc:
        with (
            tc.tile_pool(name="xres", bufs=1) as xres_pool,
            tc.tile_pool(name="xstage", bufs=2) as xstage_pool,
            tc.tile_pool(name="wstage", bufs=3) as wstage_pool,
            tc.tile_pool(name="wb", bufs=3) as wb_pool,
            tc.tile_pool(name="biasp", bufs=1) as bias_pool,
            tc.tile_pool(name="osb", bufs=4) as osb_pool,
            tc.tile_pool(name="psum", bufs=1, space="PSUM") as psum_pool,
        ):
            xr = [
                xres_pool.tile([P, K_TILES, TOK_C], mm_dt, tag=f"xr{i}", name=f"xr{i}")
                for i in range(n_x)
            ]

            def load_x_chunk(kk):
                """DMA one [128, XKG, 1024] X^T chunk and round into xr (ACT)."""
                xs = xstage_pool.tile([P, XKG, TOK_C], mybir.dt.float32, name="xs")
                nc.sync.dma_start(out=xs[:], in_=xt_v[:, kk * XKG:(kk + 1) * XKG, :])
                sl = slice(kk * XKG, (kk + 1) * XKG)
                nc.scalar.copy(out=xr[0][:, sl, :], in_=xs[:])
                if mode == "bf16x2":
                    nc.vector.tensor_sub(out=xr[1][:, sl, :], in0=xs[:], in1=xr[0][:, sl, :])

            for ob in range(O_BLOCKS):
                osl = slice(ob * OB, (ob + 1) * OB)

                # bias for this out-block, broadcast across partitions (ACT copy
                # so the DVE bias-add waits on a single semaphore)
                bstage = bias_pool.tile([P, OB], mybir.dt.float32, tag="bstage", name="bstage")
                nc.sync.dma_start(out=bstage[:], in_=bias[None, osl].to_broadcast([P, OB]))
                bias_bc = bias_pool.tile([P, OB], mybir.dt.float32, tag="bbc", name="bias_bc")
                nc.scalar.copy(out=bias_bc[:], in_=bstage[:])

                psums = [psum_pool.tile([P, OB], mybir.dt.float32, name=f"ps{_m}") for _m in range(M_TILES)]

                for kg in range(K_GROUPS):
                    if ob == 0:
                        # interleave X residency build into the first out-block
                        for kk in range(kg * KG // XKG, (kg + 1) * KG // XKG):
                            load_x_chunk(kk)
                    ws = wstage_pool.tile([P, KG, OB], mybir.dt.float32, name="ws")
                    nc.sync.dma_start(out=ws[:], in_=wt_v[:, kg * KG:(kg + 1) * KG, osl])
                    wb = wb_pool.tile([P, KG, OB], mm_dt, name="wb")
                    nc.vector.tensor_scalar(
                        out=wb[:], in0=ws[:], scalar1=0.0, scalar2=None,
                        op0=mybir.AluOpType.is_gt,
                    )
                    for ks in range(KG):
                        k = kg * KG + ks
                        for m in range(M_TILES):
                            nc.tensor.matmul(
                                out=psums[m][:],
                                lhsT=xr[0][:, k, m * P:(m + 1) * P],
                                rhs=wb[:, ks, :],
                                start=(k == 0),
                                stop=(k == K_TILES - 1) if mode != "bf16x2" else False,
                            )
                            if mode == "bf16x2":
                                nc.tensor.matmul(
                                    out=psums[m][:],
                                    lhsT=xr[1][:, k, m * P:(m + 1) * P],
                                    rhs=wb[:, ks, :],
                                    start=False,
                                    stop=(k == K_TILES - 1),
                                )

                # drain: psum -> sbuf (ACT), + bias (DVE), -> DRAM
                for m in range(M_TILES):
                    o_sb = osb_pool.tile([P, OB], mybir.dt.float32, name="o_sb")
                    nc.scalar.copy(out=o_sb[:], in_=psums[m][:])
                    nc.vector.tensor_add(out=o_sb[:], in0=o_sb[:], in1=bias_bc[:])
                    nc.sync.dma_start(out=y_v[:, m, osl], in_=o_sb[:])

    nc.compile()
    return nc


def kernel(X: np.ndarray, weight: np.ndarray, bias: np.ndarray) -> np.ndarray:
    assert X.shape == (N_TOKENS, IN_F) and weight.shape == (OUT_F, IN_F)
    mode = _MODE

    if mode not in _CACHED:
        _CACHED[mode] = build(mode)
    nc = _CACHED[mode]

    if _TRACE:
        _install_ntff_shim()

    # Host-side layout prep (sharding + transposes only; math is on-device)
    wt_np = np.ascontiguousarray(weight.T.astype(np.float32, copy=False))
    bias_np = np.ascontiguousarray(bias.astype(np.float32, copy=False))
    in_maps = []
    for c in range(N_CORES):
        xs = X[c * TOK_C:(c + 1) * TOK_C, :]
        xt_np = np.ascontiguousarray(xs.T.astype(np.float32, copy=False))
        in_maps.append({"xt": xt_np, "wt": wt_np, "bias": bias_np})

    res = run_bass_kernel_spmd(
        nc, in_maps, core_ids=list(range(N_CORES)), trace=_TRACE,
    )
    out = np.concatenate([res.results[c]["y"] for c in range(N_CORES)], axis=0)
    if _TRACE:
        kernel.last_exec_time_ns = res.exec_time_ns
        kernel.last_trace = res.instructions_and_trace
    return out.astype(np.float32, copy=False)


# revision 9
# speedup vs baseline: 1.0580x; 1.0104x over previous
"""BinaryLinear Trainium2 kernel: Y = X @ binarize(W).T + bias.

Shapes (hardcoded per the problem spec):
  X: [8192, 4096] f32, W: [4096, 4096] f32, bias: [4096] f32 -> Y: [8192, 4096] f32

Strategy: data-parallel over tokens across 8 NeuronCores (1024 tokens/core),
weight replicated. Host prepares transposed layouts (X.T shard and W.T) so the
contraction dim lands on SBUF partitions; all math (binarize + matmul + bias)
runs on-device.

Per core: X^T shard is made resident in SBUF (rounded to the compute dtype),
W^T streams through once; 2048 matmuls of [128x128]@[128x512] accumulate over
K=4096 into 8 PSUM banks per 512-wide out-block. X staging is interleaved into
the first out-block's k-loop so the PE starts ~10us in. Casts/drains run on the
otherwise-idle ACT engine; binarize + bias-add on DVE.

Compute modes (env TRNKERNEL_MODE):
  f32r   (default): fp32r matmuls — full-rate reduced-precision fp32
  bf16   : single-pass bf16 (X rounded to bf16)
  bf16x2 : X split into hi+lo bf16, two accumulating passes (near-fp32 exact)
"""
import os
import sys

import numpy as np

sys.path.insert(0, "/opt/trn_rl_repo")

import concourse.bacc as bacc
import concourse.mybir as mybir
import concourse.tile as tile
from concourse.bass_utils import run_bass_kernel_spmd

N_TOKENS = 8192
IN_F = 4096
OUT_F = 4096
N_CORES = 8
TOK_C = N_TOKENS // N_CORES  # 1024 tokens per core

P = 128
K_TILES = IN_F // P          # 32
KG = 4                       # k-tiles per W DMA/binarize group
K_GROUPS = K_TILES // KG     # 8
M_TILES = TOK_C // P         # 8
OB = 512                     # out-features per block (one PSUM bank)
O_BLOCKS = OUT_F // OB       # 8
XKG = 2                      # k-tiles per X-load DMA (1 MiB)

_MODE = os.environ.get("TRNKERNEL_MODE", "f32r")
_TRACE = os.environ.get("TRNKERNEL_TRACE", "0") == "1"

_CACHED = {}


def _install_ntff_shim():
    """Register the NTFF profile hook so trace=True yields exec_time_ns."""
    import types

    try:
        import antenv  # noqa: F401
        from trn_agent_boot.trn_boot import _ntff_profile_via_ctypes
        import concourse.bass_utils as bu

        hook = _ntff_profile_via_ctypes("/opt/axon/libaxon_pjrt.so")
        mod = types.ModuleType("antenv.axon_hooks")
        mod.get_axon_ntff_profile_hook = lambda: hook
        mod.set_axon_ntff_profile_hook = lambda h: None
        sys.modules["antenv.axon_hooks"] = mod
        bu.upload_artifacts = lambda tmpdir: tmpdir  # no artifact store here
    except Exception:
        pass


def build(mode: str):
    assert mode in ("f32r", "bf16", "bf16x2")
    mm_dt = mybir.dt.float32r if mode == "f32r" else mybir.dt.bfloat16

    nc = bacc.Bacc(None)
    xt = nc.declare_dram_parameter("xt", [IN_F, TOK_C], mybir.dt.float32, isOutput=False)
    wt = nc.declare_dram_parameter("wt", [IN_F, OUT_F], mybir.dt.float32, isOutput=False)
    bias = nc.declare_dram_parameter("bias", [OUT_F], mybir.dt.float32, isOutput=False)
    y = nc.declare_dram_parameter("y", [TOK_C, OUT_F], mybir.dt.float32, isOutput=True)

    # DRAM-side tiled views: partition dim = contraction (in-features)
    xt_v = xt.rearrange("(kt p) t -> p kt t", p=P)      # [128, 32, 1024]
    wt_v = wt.rearrange("(kt p) o -> p kt o", p=P)      # [128, 32, 4096]
    y_v = y.rearrange("(mt p) o -> p mt o", p=P)        # [128, 8, 4096]

    n_x = 2 if mode == "bf16x2" else 1

    with tile.TileContext(nc) as tc:
        with (
            tc.tile_pool(name="xres", bufs=1) as xres_pool,
            tc.tile_pool(name="xstage", bufs=2) as xstage_pool,
            tc.tile_pool(name="wstage", bufs=3) as wstage_pool,
            tc.tile_pool(name="wb", bufs=3) as wb_pool,
            tc.tile_pool(name="biasp", bufs=1) as bias_pool,
            tc.tile_pool(name="osb", bufs=4) as osb_pool,
            tc.tile_pool(name="psum", bufs=1, space="PSUM") as psum_pool,
        ):
            xr = [
                xres_pool.tile([P, K_TILES, TOK_C], mm_dt, tag=f"xr{i}", name=f"xr{i}")
                for i in range(n_x)
            ]

            def load_x_chunk(kk):
                """DMA one [128, XKG, 1024] X^T chunk and round into xr (ACT)."""
                xs = xstage_pool.tile([P, XKG, TOK_C], mybir.dt.float32, name="xs")
                nc.sync.dma_start(out=xs[:], in_=xt_v[:, kk * XKG:(kk + 1) * XKG, :])
                sl = slice(kk * XKG, (kk + 1) * XKG)
                nc.vector.tensor_scalar(
                    out=xr[0][:, sl, :], in0=xs[:], scalar1=0.0, scalar2=None,
                    op0=mybir.AluOpType.add,
                )
                if mode == "bf16x2":
                    nc.vector.tensor_sub(out=xr[1][:, sl, :], in0=xs[:], in1=xr[0][:, sl, :])

            for ob in range(O_BLOCKS):
                osl = slice(ob * OB, (ob + 1) * OB)

                # bias for this out-block, broadcast across partitions (ACT copy
                # so the DVE bias-add waits on a single semaphore)
                bstage = bias_pool.tile([P, OB], mybir.dt.float32, tag="bstage", name="bstage")
                nc.sync.dma_start(out=bstage[:], in_=bias[None, osl].to_broadcast([P, OB]))
                bias_bc = bias_pool.tile([P, OB], mybir.dt.float32, tag="bbc", name="bias_bc")
                nc.scalar.copy(out=bias_bc[:], in_=bstage[:])

                psums = [psum_pool.tile([P, OB], mybir.dt.float32, name=f"ps{_m}") for _m in range(M_TILES)]

                for kg in range(K_GROUPS):
                    if ob == 0:
                        # interleave X residency build into the first out-block
                        for kk in range(kg * KG // XKG, (kg + 1) * KG // XKG):
                            load_x_chunk(kk)
                    ws = wstage_pool.tile([P, KG, OB], mybir.dt.float32, name="ws")
                    nc.sync.dma_start(out=ws[:], in_=wt_v[:, kg * KG:(kg + 1) * KG, osl])
                    wb = wb_pool.tile([P, KG, OB], mm_dt, name="wb")
                    nc.vector.tensor_scalar(
                        out=wb[:], in0=ws[:], scalar1=0.0, scalar2=None,
                        op0=mybir.AluOpType.is_gt,
                    )
                    for ks in range(KG):
                        k = kg * KG + ks
                        for m in range(M_TILES):
                            nc.tensor.matmul(
                                out=psums[m][:],
                                lhsT=xr[0][:, k, m * P:(m + 1) * P],
                                rhs=wb[:, ks, :],
                                start=(k == 0),
                                stop=(k == K_TILES - 1) if mode != "bf16x2" else False,
                            )
                            if mode == "bf16x2":
                                nc.tensor.matmul(
                                    out=psums[m][:],
                                    lhsT=xr[1][:, k, m * P:(m + 1) * P],
                                    rhs=wb[:, ks, :],
                                    start=False,
                                    stop=(k == K_TILES - 1),
                                )

                # drain: psum -> sbuf (ACT), + bias (DVE), -> DRAM
                for m in range(M_TILES):
                    o_sb = osb_pool.tile([P, OB], mybir.dt.float32, name="o_sb")
                    nc.scalar.copy(out=o_sb[:], in_=psums[m][:])
                    nc.vector.tensor_add(out=o_sb[:], in0=o_sb[:], in1=bias_bc[:])
                    nc.sync.dma_start(out=y_v[:, m, osl], in_=o_sb[:])

    nc.compile()
    return nc


def kernel(X: np.ndarray, weight: np.ndarray, bias: np.ndarray) -> np.ndarray:
    assert X.shape == (N_TOKENS, IN_F) and weight.shape == (OUT_F, IN_F)
    mode = _MODE

    if mode not in _CACHED:
        _CACHED[mode] = build(mode)
    nc = _CACHED[mode]

    if _TRACE:
        _install_ntff_shim()

    # Host-side layout prep (sharding + transposes only; math is on-device)
    wt_np = np.ascontiguousarray(weight.T.astype(np.float32, copy=False))
    bias_np = np.ascontiguousarray(bias.astype(np.float32, copy=False))
    in_maps = []
    for c in range(N_CORES):
        xs = X[c * TOK_C:(c + 1) * TOK_C, :]
        xt_np = np.ascontiguousarray(xs.T.astype(np.float32, copy=False))
        in_maps.append({"xt": xt_np, "wt": wt_np, "bias": bias_np})

    res = run_bass_kernel_spmd(
        nc, in_maps, core_ids=list(range(N_CORES)), trace=_TRACE,
    )
    out = np.concatenate([res.results[c]["y"] for c in range(N_CORES)], axis=0)
    if _TRACE:
        kernel.last_exec_time_ns = res.exec_time_ns
        kernel.last_trace = res.instructions_and_trace
    return out.astype(np.float32, copy=False)


# revision 10
# speedup vs baseline: 1.0582x; 1.0002x over previous
"""BinaryLinear Trainium2 kernel: Y = X @ binarize(W).T + bias.

Shapes (hardcoded per the problem spec):
  X: [8192, 4096] f32, W: [4096, 4096] f32, bias: [4096] f32 -> Y: [8192, 4096] f32

Strategy: data-parallel over tokens across 8 NeuronCores (1024 tokens/core),
weight replicated. Host prepares transposed layouts (X.T shard and W.T) so the
contraction dim lands on SBUF partitions; all math (binarize + matmul + bias)
runs on-device.

Per core: X^T shard is made resident in SBUF (rounded to the compute dtype),
W^T streams through once; 2048 matmuls of [128x128]@[128x512] accumulate over
K=4096 into 8 PSUM banks per 512-wide out-block. X staging is interleaved into
the first out-block's k-loop so the PE starts ~10us in. Casts/drains run on the
otherwise-idle ACT engine; binarize + bias-add on DVE.

Compute modes (env TRNKERNEL_MODE):
  f32r   (default): fp32r matmuls — full-rate reduced-precision fp32
  bf16   : single-pass bf16 (X rounded to bf16)
  bf16x2 : X split into hi+lo bf16, two accumulating passes (near-fp32 exact)
"""
import os
import sys

import numpy as np

sys.path.insert(0, "/opt/trn_rl_repo")

import concourse.bacc as bacc
import concourse.mybir as mybir
import concourse.tile as tile
from concourse.bass_utils import run_bass_kernel_spmd

N_TOKENS = 8192
IN_F = 4096
OUT_F = 4096
N_CORES = 8
TOK_C = N_TOKENS // N_CORES  # 1024 tokens per core

P = 128
K_TILES = IN_F // P          # 32
KG = 4                       # k-tiles per W DMA/binarize group
K_GROUPS = K_TILES // KG     # 8
M_TILES = TOK_C // P         # 8
OB = 512                     # out-features per block (one PSUM bank)
O_BLOCKS = OUT_F // OB       # 8
XKG = 2                      # k-tiles per X-load DMA (1 MiB)

_MODE = os.environ.get("TRNKERNEL_MODE", "f32r")
_TRACE = os.environ.get("TRNKERNEL_TRACE", "0") == "1"

_CACHED = {}


def _install_ntff_shim():
    """Register the NTFF profile hook so trace=True yields exec_time_ns."""
    import types

    try:
        import antenv  # noqa: F401
        from trn_agent_boot.trn_boot import _ntff_profile_via_ctypes
        import concourse.bass_utils as bu

        hook = _ntff_profile_via_ctypes("/opt/axon/libaxon_pjrt.so")
        mod = types.ModuleType("antenv.axon_hooks")
        mod.get_axon_ntff_profile_hook = lambda: hook
        mod.set_axon_ntff_profile_hook = lambda h: None
        sys.modules["antenv.axon_hooks"] = mod
        bu.upload_artifacts = lambda tmpdir: tmpdir  # no artifact store here
    except Exception:
        pass


def build(mode: str):
    assert mode in ("f32r", "bf16", "bf16x2")
    mm_dt = mybir.dt.float32r if mode == "f32r" else mybir.dt.bfloat16

    nc = bacc.Bacc(None)
    xt = nc.declare_dram_parameter("xt", [IN_F, TOK_C], mybir.dt.float32, isOutput=False)
    wt = nc.declare_dram_parameter("wt", [IN_F, OUT_F], mybir.dt.float32, isOutput=False)
    bias = nc.declare_dram_parameter("bias", [OUT_F], mybir.dt.float32, isOutput=False)
    y = nc.declare_dram_parameter("y", [TOK_C, OUT_F], mybir.dt.float32, isOutput=True)

    # DRAM-side tiled views: partition dim = contraction (in-features)
    xt_v = xt.rearrange("(kt p) t -> p kt t", p=P)      # [128, 32, 1024]
    wt_v = wt.rearrange("(kt p) o -> p kt o", p=P)      # [128, 32, 4096]
    y_v = y.rearrange("(mt p) o -> p mt o", p=P)        # [128, 8, 4096]

    n_x = 2 if mode == "bf16x2" else 1

    with tile.TileContext(nc) as tc:
        with (
            tc.tile_pool(name="xres", bufs=1) as xres_pool,
            tc.tile_pool(name="xstage", bufs=2) as xstage_pool,
            tc.tile_pool(name="wstage", bufs=3) as wstage_pool,
            tc.tile_pool(name="wb", bufs=3) as wb_pool,
            tc.tile_pool(name="biasp", bufs=1) as bias_pool,
            tc.tile_pool(name="osb", bufs=4) as osb_pool,
            tc.tile_pool(name="psum", bufs=1, space="PSUM") as psum_pool,
        ):
            xr = [
                xres_pool.tile([P, K_TILES, TOK_C], mm_dt, tag=f"xr{i}", name=f"xr{i}")
                for i in range(n_x)
            ]

            def load_x_chunk(kk):
                """DMA one [128, XKG, 1024] X^T chunk and round into xr (ACT)."""
                xs = xstage_pool.tile([P, XKG, TOK_C], mybir.dt.float32, name="xs")
                nc.sync.dma_start(out=xs[:], in_=xt_v[:, kk * XKG:(kk + 1) * XKG, :])
                sl = slice(kk * XKG, (kk + 1) * XKG)
                nc.vector.tensor_scalar(
                    out=xr[0][:, sl, :], in0=xs[:], scalar1=0.0, scalar2=None,
                    op0=mybir.AluOpType.add,
                )
                if mode == "bf16x2":
                    nc.vector.tensor_sub(out=xr[1][:, sl, :], in0=xs[:], in1=xr[0][:, sl, :])

            for ob in range(O_BLOCKS):
                osl = slice(ob * OB, (ob + 1) * OB)

                # bias for this out-block, broadcast across partitions (ACT copy
                # so the DVE bias-add waits on a single semaphore)
                bstage = bias_pool.tile([P, OB], mybir.dt.float32, tag="bstage", name="bstage")
                nc.sync.dma_start(out=bstage[:], in_=bias[None, osl].to_broadcast([P, OB]))
                bias_bc = bias_pool.tile([P, OB], mybir.dt.float32, tag="bbc", name="bias_bc")
                nc.scalar.copy(out=bias_bc[:], in_=bstage[:])

                psums = [psum_pool.tile([P, OB], mybir.dt.float32, name=f"ps{_m}") for _m in range(M_TILES)]

                for kg in range(K_GROUPS):
                    if ob == 0:
                        # interleave X residency build into the first out-block;
                        # first chunk ahead of the W slab so MM k=0 unblocks early
                        load_x_chunk(kg * 2)
                    ws = wstage_pool.tile([P, KG, OB], mybir.dt.float32, name="ws")
                    nc.sync.dma_start(out=ws[:], in_=wt_v[:, kg * KG:(kg + 1) * KG, osl])
                    if ob == 0:
                        load_x_chunk(kg * 2 + 1)
                    wb = wb_pool.tile([P, KG, OB], mm_dt, name="wb")
                    nc.vector.tensor_scalar(
                        out=wb[:], in0=ws[:], scalar1=0.0, scalar2=None,
                        op0=mybir.AluOpType.is_gt,
                    )
                    for ks in range(KG):
                        k = kg * KG + ks
                        for m in range(M_TILES):
                            nc.tensor.matmul(
                                out=psums[m][:],
                                lhsT=xr[0][:, k, m * P:(m + 1) * P],
                                rhs=wb[:, ks, :],
                                start=(k == 0),
                                stop=(k == K_TILES - 1) if mode != "bf16x2" else False,
                            )
                            if mode == "bf16x2":
                                nc.tensor.matmul(
                                    out=psums[m][:],
                                    lhsT=xr[1][:, k, m * P:(m + 1) * P],
                                    rhs=wb[:, ks, :],
                                    start=False,
                                    stop=(k == K_TILES - 1),
                                )

                # drain: psum -> sbuf (ACT), + bias (DVE), -> DRAM
                for m in range(M_TILES):
                    o_sb = osb_pool.tile([P, OB], mybir.dt.float32, name="o_sb")
                    nc.scalar.copy(out=o_sb[:], in_=psums[m][:])
                    nc.vector.tensor_add(out=o_sb[:], in0=o_sb[:], in1=bias_bc[:])
                    nc.sync.dma_start(out=y_v[:, m, osl], in_=o_sb[:])

    nc.compile()
    return nc


def kernel(X: np.ndarray, weight: np.ndarray, bias: np.ndarray) -> np.ndarray:
    assert X.shape == (N_TOKENS, IN_F) and weight.shape == (OUT_F, IN_F)
    mode = _MODE

    if mode not in _CACHED:
        _CACHED[mode] = build(mode)
    nc = _CACHED[mode]

    if _TRACE:
        _install_ntff_shim()

    # Host-side layout prep (sharding + transposes only; math is on-device)
    wt_np = np.ascontiguousarray(weight.T.astype(np.float32, copy=False))
    bias_np = np.ascontiguousarray(bias.astype(np.float32, copy=False))
    in_maps = []
    for c in range(N_CORES):
        xs = X[c * TOK_C:(c + 1) * TOK_C, :]
        xt_np = np.ascontiguousarray(xs.T.astype(np.float32, copy=False))
        in_maps.append({"xt": xt_np, "wt": wt_np, "bias": bias_np})

    res = run_bass_kernel_spmd(
        nc, in_maps, core_ids=list(range(N_CORES)), trace=_TRACE,
    )
    out = np.concatenate([res.results[c]["y"] for c in range(N_CORES)], axis=0)
    if _TRACE:
        kernel.last_exec_time_ns = res.exec_time_ns
        kernel.last_trace = res.instructions_and_trace
    return out.astype(np.float32, copy=False)
